# revision 15
# baseline (speedup 1.0000x reference)
"""Trainium2 Bass kernel for nn_AttentionBlock (gnn_message_passing).

Key simplification: the reference softmax is over a size-1 axis, so the
attention weights are exactly 1.0 and the patch einsum collapses to a sum
over all 1024 patches.  The whole module reduces to:

  S[b,c,p1,p2] = sum_{i,j} feature[b,c,16i+p1,16j+p2]        (201 MB read)
  u[b] = S[b] . W_patch (permuted) + 1024*b_patch            [256]
  v[b] = u[b] @ W                                            [512]
  y[b] = 0.25*(v[b] @ W_out + b_out) + 0.75*token[b]
  out[b] = layernorm(y[b]) * gamma + beta

Sharding: pure data parallel, batch 64 -> 8 cores x 8 batches.  Each core
streams its 25 MB feature shard (DMA-bound), reduces (k, j) on the vector
engine, collapses the remaining cross-partition i-dim with a 0/1 matmul,
and runs the small matmul tail in float32r (single-pass fp32, ~1e-4).
"""

import numpy as np
from contextlib import ExitStack

import concourse.bass as bass
import concourse.tile as tile
from concourse import bacc, mybir
from concourse.bass_utils import run_bass_kernel_spmd

F32 = mybir.dt.float32
F32R = mybir.dt.float32r

N_CORES = 8
B = 64
C = 3
H = 512
W_IMG = 512
P = 16                      # patch size
IN_F = 256
HD = 512
BB = B // N_CORES           # 8 batches per core
EPS = 1e-5


def _build_kernel_body(tc):
    nc = tc.nc
    feat = nc.dram_tensor("feature", [BB, C, H, W_IMG], F32R, kind="ExternalInput").ap()
    tok_adj = nc.dram_tensor("tok_adj", [BB, IN_F], F32, kind="ExternalInput").ap()
    gb = nc.dram_tensor("gb", [BB, 2 * IN_F], F32, kind="ExternalInput").ap()
    sel8 = nc.dram_tensor("sel8", [128, 16], F32R, kind="ExternalInput").ap()
    ident8 = nc.dram_tensor("ident8", [BB, BB], F32, kind="ExternalInput").ap()
    w_pp = nc.dram_tensor("w_pp", [16, C * 16 * IN_F], F32R, kind="ExternalInput").ap()
    w_mid = nc.dram_tensor("w_mid", [128, 2 * HD], F32R, kind="ExternalInput").ap()
    w_out = nc.dram_tensor("w_out", [128, 4 * IN_F], F32R, kind="ExternalInput").ap()
    out = nc.dram_tensor("out", [BB, IN_F], F32, kind="ExternalOutput").ap()

    with ExitStack() as ctx:
        mega = ctx.enter_context(tc.tile_pool(name="mega", bufs=12))
        small = ctx.enter_context(tc.tile_pool(name="small", bufs=1))
        work = ctx.enter_context(tc.tile_pool(name="work", bufs=2))
        psum = ctx.enter_context(tc.tile_pool(name="psum", bufs=4, space="PSUM"))
        psum_t = ctx.enter_context(tc.tile_pool(name="psum_t", bufs=1, space="PSUM"))
        psum_u = ctx.enter_context(tc.tile_pool(name="psum_u", bufs=1, space="PSUM"))
        psum_mm = ctx.enter_context(tc.tile_pool(name="psum_mm", bufs=1, space="PSUM"))

        # ---- constants / weights ----
        w_pp_t = small.tile([16, C * 16 * IN_F], F32R)
        nc.sync.dma_start(w_pp_t, w_pp)
        w_mid_t = small.tile([128, 2 * HD], F32R)
        nc.sync.dma_start(w_mid_t, w_mid)
        w_out_t = small.tile([128, 4 * IN_F], F32R)
        nc.sync.dma_start(w_out_t, w_out)
        gb_t = small.tile([BB, 2 * IN_F], F32)
        nc.sync.dma_start(gb_t, gb)
        tok_t = small.tile([BB, IN_F], F32)
        nc.sync.dma_start(tok_t, tok_adj)
        sel8_t = small.tile([128, 16], F32R)
        nc.sync.dma_start(sel8_t, sel8)
        ident8_t = small.tile([BB, BB], F32)
        nc.sync.dma_start(ident8_t, ident8)
        eps_t = small.tile([BB, 1], F32)
        nc.vector.memset(eps_t, EPS)

        # ---- stage 1 + 2 + 3, channel-major so the tail pipelines ----
        # per (b, c): one 3-D DMA, full 128 partitions, 1 MB (f32r):
        #   sbuf [r=128 rows (i8, p1), (k4, w512)];  row r_full = k*128 + r
        # PE collapses (i8, k) with four f32r sel8 matmuls accumulating in
        # PSUM [p1, (j, q)]; DVE then reduces j (strided) into st32_all.
        st32_all = small.tile([16, BB * C * 16], F32)
        u_ps = psum_u.tile([BB, IN_F], F32, tag="u")
        dma_engines = [nc.sync, nc.scalar]
        for c in range(C):
            for b in range(BB):
                idx = c * BB + b
                # rows 0-255 (k0,k1) land normally; rows 256-511 (k2,k3) are
                # DMA-accumulated onto them by the SWDGE CCE adders.
                mt = mega.tile([128, 2 * W_IMG], F32R)
                mtv = mt[:].rearrange("p (k w) -> p k w", w=W_IMG)
                src_lo = feat[b, c, 0:2 * 128, :].rearrange("(k r) w -> r k w", r=128)
                src_hi = feat[b, c, 2 * 128:4 * 128, :].rearrange("(k r) w -> r k w", r=128)
                dma_engines[idx % 2].dma_start(mtv, src_lo)
                nc.gpsimd.dma_start(mtv, src_hi, accum_op=mybir.AluOpType.add)
                # DVE folds the remaining k-pair (f32r out), PE contracts i8
                t01 = work.tile([128, W_IMG], F32R, tag="t01", bufs=4)
                nc.vector.tensor_add(t01, mt[:, 0:W_IMG], mt[:, W_IMG:2 * W_IMG])
                bc_ps = psum.tile([16, W_IMG], F32, tag="bc")
                nc.tensor.matmul(bc_ps, sel8_t, t01, start=True, stop=True)
                # bc_ps free index = j*16 + q ; reduce j, keep q
                mv = bc_ps[:].rearrange("p (j q) -> p q j", q=16)
                nc.vector.reduce_sum(
                    st32_all[:, (b * C + c) * 16:(b * C + c + 1) * 16],
                    mv,
                    axis=mybir.AxisListType.X,
                )

            # ---- per channel: cast the c-slice to f32r, u matmuls ----
            st_sb = small.tile([16, 128], F32R, tag=f"st{c}")
            stview = st32_all[:].rearrange("p (b c q) -> p b c q", c=C, q=16)
            nc.vector.tensor_copy(
                st_sb[:].rearrange("p (b q) -> p b q", q=16), stview[:, :, c, :]
            )

            stv = st_sb[:].rearrange("k (b q) -> k b q", q=16)
            for p2 in range(P):
                nc.tensor.matmul(
                    u_ps,
                    stv[:, :, p2],
                    w_pp_t[:, (c * P + p2) * IN_F:(c * P + p2 + 1) * IN_F],
                    start=(c == 0 and p2 == 0),
                    stop=(c == C - 1 and p2 == P - 1),
                )

        # ---- stage 4: transpose u to [256, 8] (bias folded into tok_adj) ----
        u_sb = work.tile([BB, IN_F], F32)
        nc.scalar.copy(u_sb, u_ps)

        uT_sb = work.tile([128, 2 * BB], F32R)
        for h in range(2):
            ut_ps = psum_t.tile([128, BB], F32, tag="tp2")
            nc.tensor.transpose(ut_ps, u_sb[:, h * 128:(h + 1) * 128], ident8_t)
            nc.vector.tensor_copy(uT_sb[:, h * BB:(h + 1) * BB], ut_ps)

        # ---- stage 5: v = u @ W ----
        v_ps = psum_mm.tile([BB, HD], F32, tag="v")
        for h in range(2):
            nc.tensor.matmul(
                v_ps,
                uT_sb[:, h * BB:(h + 1) * BB],
                w_mid_t[:, h * HD:(h + 1) * HD],
                start=(h == 0),
                stop=(h == 1),
            )
        v_sb = work.tile([BB, HD], F32)
        nc.scalar.copy(v_sb, v_ps)

        vT_sb = work.tile([128, 4 * BB], F32R)
        for q in range(4):
            vt_ps = psum_t.tile([128, BB], F32, tag="tp2")
            nc.tensor.transpose(vt_ps, v_sb[:, q * 128:(q + 1) * 128], ident8_t)
            nc.vector.tensor_copy(vT_sb[:, q * BB:(q + 1) * BB], vt_ps)

        # ---- stage 6: y = 0.25*v@W_out + (0.25*b_out + 0.75*token) ----
        y_ps = psum_mm.tile([BB, IN_F], F32, tag="y")
        for q in range(4):
            nc.tensor.matmul(
                y_ps,
                vT_sb[:, q * BB:(q + 1) * BB],
                w_out_t[:, q * IN_F:(q + 1) * IN_F],
                start=(q == 0),
                stop=(q == 3),
            )
        y_sb = work.tile([BB, IN_F], F32)
        nc.vector.tensor_add(y_sb, y_ps, tok_t)

        # ---- stage 7: layernorm ----
        stats = work.tile([BB, 6], F32)
        nc.vector.bn_stats(stats, y_sb)
        mv2 = work.tile([BB, 2], F32)
        nc.vector.bn_aggr(mv2, stats)
        std = work.tile([BB, 1], F32)
        nc.scalar.activation(std, mv2[:, 1:2], mybir.ActivationFunctionType.Sqrt,
                             bias=eps_t, scale=1.0)
        rstd = work.tile([BB, 1], F32)
        nc.vector.reciprocal(rstd, std)
        xm = work.tile([BB, IN_F], F32)
        nc.vector.tensor_scalar(xm, y_sb, mv2[:, 0:1], rstd,
                                op0=mybir.AluOpType.subtract,
                                op1=mybir.AluOpType.mult)
        out_sb = work.tile([BB, IN_F], F32)
        nc.vector.tensor_mul(out_sb, xm, gb_t[:, 0:IN_F])
        nc.vector.tensor_add(out_sb, out_sb, gb_t[:, IN_F:2 * IN_F])
        nc.sync.dma_start(out, out_sb)


_CACHE = {}


def _get_program():
    if "nc" not in _CACHE:
        nc = bacc.Bacc("TRN2", target_bir_lowering=False, debug=False,
                       num_devices=N_CORES)
        with tile.TileContext(nc) as tc:
            _build_kernel_body(tc)
        nc.compile()
        _CACHE["nc"] = nc
    return _CACHE["nc"]


def _prep_weights(W_patch, b_patch, W, W_out, b_out, gamma, beta):
    # w_pp[p1, (c, p2, f)] = W_patch[(p1*16+p2)*3 + c, f]
    wp4 = W_patch.reshape(P, P, C, IN_F).transpose(0, 2, 1, 3)   # [p1, c, p2, f]
    w_pp = np.ascontiguousarray(wp4.reshape(P, C * P * IN_F), dtype=np.float32)
    sel8 = np.ascontiguousarray(
        np.tile(np.eye(P, dtype=np.float32), (8, 1)))            # [128, 16]
    w_mid = np.ascontiguousarray(
        np.concatenate([W[0:128, :], W[128:256, :]], axis=1), dtype=np.float32
    )
    wo = 0.25 * W_out
    w_out_t = np.ascontiguousarray(
        np.concatenate([wo[q * 128:(q + 1) * 128, :] for q in range(4)], axis=1),
        dtype=np.float32,
    )
    gb = np.ascontiguousarray(
        np.tile(np.concatenate([gamma, beta])[None, :], (BB, 1)), dtype=np.float32
    )
    ident8 = np.ascontiguousarray(np.eye(BB), dtype=np.float32)
    return w_pp, w_mid, w_out_t, gb, ident8, sel8


def kernel(**inputs):
    feature = np.asarray(inputs["feature"], dtype=np.float32)
    token = np.asarray(inputs["token"], dtype=np.float32)
    b_out = np.asarray(inputs["b_out"], dtype=np.float32)
    w_pp, w_mid, w_out_t, gb, ident8, sel8 = _prep_weights(
        np.asarray(inputs["W_patch"], dtype=np.float32),
        np.asarray(inputs["b_patch"], dtype=np.float32),
        np.asarray(inputs["W"], dtype=np.float32),
        np.asarray(inputs["W_out"], dtype=np.float32),
        b_out,
        np.asarray(inputs["gamma"], dtype=np.float32),
        np.asarray(inputs["beta"], dtype=np.float32),
    )
    W_full = np.asarray(inputs["W"], dtype=np.float32)
    W_out_full = np.asarray(inputs["W_out"], dtype=np.float32)
    b_patch_f = np.asarray(inputs["b_patch"], dtype=np.float32)
    bias_path = 0.25 * ((1024.0 * b_patch_f) @ W_full @ W_out_full)
    tok_adj = (0.75 * token + 0.25 * b_out[None, :] + bias_path[None, :]).astype(np.float32)
    nc = _get_program()
    in_maps = []
    for i in range(N_CORES):
        in_maps.append({
            "feature": np.ascontiguousarray(feature[i * BB:(i + 1) * BB]),
            "tok_adj": np.ascontiguousarray(tok_adj[i * BB:(i + 1) * BB]),
            "gb": gb,
            "sel8": sel8,
            "ident8": ident8,
            "w_pp": w_pp,
            "w_mid": w_mid,
            "w_out": w_out_t,
        })
    res = run_bass_kernel_spmd(nc, in_maps, list(range(N_CORES))).results
    return np.ascontiguousarray(
        np.concatenate([res[i]["out"] for i in range(N_CORES)], axis=0),
        dtype=np.float32,
    )


# revision 16
# speedup vs baseline: 1.2535x; 1.2535x over previous
"""Trainium2 Bass kernel for nn_AttentionBlock (gnn_message_passing).

Key simplification: the reference softmax is over a size-1 axis, so the
attention weights are exactly 1.0 and the patch einsum collapses to a sum
over all 1024 patches.  The whole module reduces to:

  S[b,c,p1,p2] = sum_{i,j} feature[b,c,16i+p1,16j+p2]        (201 MB read)
  u[b] = S[b] . W_patch (permuted) + 1024*b_patch            [256]
  v[b] = u[b] @ W                                            [512]
  y[b] = 0.25*(v[b] @ W_out + b_out) + 0.75*token[b]
  out[b] = layernorm(y[b]) * gamma + beta

Sharding: pure data parallel, batch 64 -> 8 cores x 8 batches.  Each core
streams its 25 MB feature shard (DMA-bound), reduces (k, j) on the vector
engine, collapses the remaining cross-partition i-dim with a 0/1 matmul,
and runs the small matmul tail in float32r (single-pass fp32, ~1e-4).
"""

import numpy as np
from contextlib import ExitStack

import concourse.bass as bass
import concourse.tile as tile
from concourse import bacc, mybir
from concourse.bass_utils import run_bass_kernel_spmd

F32 = mybir.dt.float32
F32R = mybir.dt.float32r

N_CORES = 8
B = 64
C = 3
H = 512
W_IMG = 512
P = 16                      # patch size
IN_F = 256
HD = 512
BB = B // N_CORES           # 8 batches per core
EPS = 1e-5


def _build_kernel_body(tc):
    nc = tc.nc
    feat = nc.dram_tensor("feature", [BB, C, H, W_IMG], F32R, kind="ExternalInput").ap()
    tok_adj = nc.dram_tensor("tok_adj", [BB, IN_F], F32, kind="ExternalInput").ap()
    gb = nc.dram_tensor("gb", [BB, 2 * IN_F], F32, kind="ExternalInput").ap()
    sel8 = nc.dram_tensor("sel8", [128, 16], F32R, kind="ExternalInput").ap()
    ident8 = nc.dram_tensor("ident8", [BB, BB], F32, kind="ExternalInput").ap()
    w_pp = nc.dram_tensor("w_pp", [16, C * 16 * IN_F], F32R, kind="ExternalInput").ap()
    w_mid = nc.dram_tensor("w_mid", [128, 2 * HD], F32R, kind="ExternalInput").ap()
    w_out = nc.dram_tensor("w_out", [128, 4 * IN_F], F32R, kind="ExternalInput").ap()
    out = nc.dram_tensor("out", [BB, IN_F], F32, kind="ExternalOutput").ap()

    with ExitStack() as ctx:
        mega = ctx.enter_context(tc.tile_pool(name="mega", bufs=8))
        small = ctx.enter_context(tc.tile_pool(name="small", bufs=1))
        work = ctx.enter_context(tc.tile_pool(name="work", bufs=2))
        psum = ctx.enter_context(tc.tile_pool(name="psum", bufs=4, space="PSUM"))
        psum_t = ctx.enter_context(tc.tile_pool(name="psum_t", bufs=1, space="PSUM"))
        psum_u = ctx.enter_context(tc.tile_pool(name="psum_u", bufs=1, space="PSUM"))
        psum_mm = ctx.enter_context(tc.tile_pool(name="psum_mm", bufs=1, space="PSUM"))

        # ---- constants / weights ----
        w_pp_t = small.tile([16, C * 16 * IN_F], F32R)
        nc.sync.dma_start(w_pp_t, w_pp)
        w_mid_t = small.tile([128, 2 * HD], F32R)
        nc.sync.dma_start(w_mid_t, w_mid)
        w_out_t = small.tile([128, 4 * IN_F], F32R)
        nc.sync.dma_start(w_out_t, w_out)
        gb_t = small.tile([BB, 2 * IN_F], F32)
        nc.sync.dma_start(gb_t, gb)
        tok_t = small.tile([BB, IN_F], F32)
        nc.sync.dma_start(tok_t, tok_adj)
        sel8_t = small.tile([128, 16], F32R)
        nc.sync.dma_start(sel8_t, sel8)
        ident8_t = small.tile([BB, BB], F32)
        nc.sync.dma_start(ident8_t, ident8)
        eps_t = small.tile([BB, 1], F32)
        nc.vector.memset(eps_t, EPS)

        # ---- stage 1 + 2 + 3, channel-major so the tail pipelines ----
        # per (b, c): one 3-D DMA, full 128 partitions, 1 MB (f32r):
        #   sbuf [r=128 rows (i8, p1), (k4, w512)];  row r_full = k*128 + r
        # PE collapses (i8, k) with four f32r sel8 matmuls accumulating in
        # PSUM [p1, (j, q)]; DVE then reduces j (strided) into st32_all.
        st32_all = small.tile([16, BB * C * 16], F32)
        u_ps = psum_u.tile([BB, IN_F], F32, tag="u")
        dma_engines = [nc.sync, nc.scalar]
        for c in range(C):
            for b in range(BB):
                idx = c * BB + b
                mt = mega.tile([128, 4 * W_IMG], F32R)
                src = feat[b, c].rearrange("(k r) w -> r k w", r=128)
                dma_engines[idx % 2].dma_start(
                    mt[:].rearrange("p (k w) -> p k w", w=W_IMG), src
                )
                # DVE folds k-pairs (f32r out), PE contracts i8 via sel8.
                # Alternate DVE-heavy and PE-heavy pairs so both engines stay
                # below the DMA delivery rate (no end-of-stream backlog).
                t01 = work.tile([128, W_IMG], F32R, tag="t01", bufs=4)
                nc.vector.tensor_add(t01, mt[:, 0:W_IMG], mt[:, W_IMG:2 * W_IMG])
                bc_ps = psum.tile([16, W_IMG], F32, tag="bc")
                if idx % 2 == 0:
                    t23 = work.tile([128, W_IMG], F32R, tag="t23", bufs=4)
                    nc.vector.tensor_add(t23, mt[:, 2 * W_IMG:3 * W_IMG],
                                         mt[:, 3 * W_IMG:4 * W_IMG])
                    nc.tensor.matmul(bc_ps, sel8_t, t01, start=True, stop=False)
                    nc.tensor.matmul(bc_ps, sel8_t, t23, start=False, stop=True)
                else:
                    nc.tensor.matmul(bc_ps, sel8_t, t01, start=True, stop=False)
                    nc.tensor.matmul(bc_ps, sel8_t,
                                     mt[:, 2 * W_IMG:3 * W_IMG],
                                     start=False, stop=False)
                    nc.tensor.matmul(bc_ps, sel8_t,
                                     mt[:, 3 * W_IMG:4 * W_IMG],
                                     start=False, stop=True)
                # bc_ps free index = j*16 + q ; reduce j, keep q
                mv = bc_ps[:].rearrange("p (j q) -> p q j", q=16)
                nc.vector.reduce_sum(
                    st32_all[:, (b * C + c) * 16:(b * C + c + 1) * 16],
                    mv,
                    axis=mybir.AxisListType.X,
                )

            # ---- per channel: cast the c-slice to f32r, u matmuls ----
            st_sb = small.tile([16, 128], F32R, tag=f"st{c}")
            stview = st32_all[:].rearrange("p (b c q) -> p b c q", c=C, q=16)
            nc.vector.tensor_copy(
                st_sb[:].rearrange("p (b q) -> p b q", q=16), stview[:, :, c, :]
            )

            stv = st_sb[:].rearrange("k (b q) -> k b q", q=16)
            for p2 in range(P):
                nc.tensor.matmul(
                    u_ps,
                    stv[:, :, p2],
                    w_pp_t[:, (c * P + p2) * IN_F:(c * P + p2 + 1) * IN_F],
                    start=(c == 0 and p2 == 0),
                    stop=(c == C - 1 and p2 == P - 1),
                )

        # ---- stage 4: transpose u to [256, 8] (bias folded into tok_adj) ----
        u_sb = work.tile([BB, IN_F], F32)
        nc.scalar.copy(u_sb, u_ps)

        uT_sb = work.tile([128, 2 * BB], F32R)
        for h in range(2):
            ut_ps = psum_t.tile([128, BB], F32, tag="tp2")
            nc.tensor.transpose(ut_ps, u_sb[:, h * 128:(h + 1) * 128], ident8_t)
            nc.vector.tensor_copy(uT_sb[:, h * BB:(h + 1) * BB], ut_ps)

        # ---- stage 5: v = u @ W ----
        v_ps = psum_mm.tile([BB, HD], F32, tag="v")
        for h in range(2):
            nc.tensor.matmul(
                v_ps,
                uT_sb[:, h * BB:(h + 1) * BB],
                w_mid_t[:, h * HD:(h + 1) * HD],
                start=(h == 0),
                stop=(h == 1),
            )
        v_sb = work.tile([BB, HD], F32)
        nc.scalar.copy(v_sb, v_ps)

        vT_sb = work.tile([128, 4 * BB], F32R)
        for q in range(4):
            vt_ps = psum_t.tile([128, BB], F32, tag="tp2")
            nc.tensor.transpose(vt_ps, v_sb[:, q * 128:(q + 1) * 128], ident8_t)
            nc.vector.tensor_copy(vT_sb[:, q * BB:(q + 1) * BB], vt_ps)

        # ---- stage 6: y = 0.25*v@W_out + (0.25*b_out + 0.75*token) ----
        y_ps = psum_mm.tile([BB, IN_F], F32, tag="y")
        for q in range(4):
            nc.tensor.matmul(
                y_ps,
                vT_sb[:, q * BB:(q + 1) * BB],
                w_out_t[:, q * IN_F:(q + 1) * IN_F],
                start=(q == 0),
                stop=(q == 3),
            )
        y_sb = work.tile([BB, IN_F], F32)
        nc.vector.tensor_add(y_sb, y_ps, tok_t)

        # ---- stage 7: layernorm ----
        stats = work.tile([BB, 6], F32)
        nc.vector.bn_stats(stats, y_sb)
        mv2 = work.tile([BB, 2], F32)
        nc.vector.bn_aggr(mv2, stats)
        std = work.tile([BB, 1], F32)
        nc.scalar.activation(std, mv2[:, 1:2], mybir.ActivationFunctionType.Sqrt,
                             bias=eps_t, scale=1.0)
        rstd = work.tile([BB, 1], F32)
        nc.vector.reciprocal(rstd, std)
        xm = work.tile([BB, IN_F], F32)
        nc.vector.tensor_scalar(xm, y_sb, mv2[:, 0:1], rstd,
                                op0=mybir.AluOpType.subtract,
                                op1=mybir.AluOpType.mult)
        out_sb = work.tile([BB, IN_F], F32)
        nc.vector.tensor_mul(out_sb, xm, gb_t[:, 0:IN_F])
        nc.vector.tensor_add(out_sb, out_sb, gb_t[:, IN_F:2 * IN_F])
        nc.sync.dma_start(out, out_sb)


_CACHE = {}


def _get_program():
    if "nc" not in _CACHE:
        nc = bacc.Bacc("TRN2", target_bir_lowering=False, debug=False,
                       num_devices=N_CORES)
        with tile.TileContext(nc) as tc:
            _build_kernel_body(tc)
        nc.compile()
        _CACHE["nc"] = nc
    return _CACHE["nc"]


def _prep_weights(W_patch, b_patch, W, W_out, b_out, gamma, beta):
    # w_pp[p1, (c, p2, f)] = W_patch[(p1*16+p2)*3 + c, f]
    wp4 = W_patch.reshape(P, P, C, IN_F).transpose(0, 2, 1, 3)   # [p1, c, p2, f]
    w_pp = np.ascontiguousarray(wp4.reshape(P, C * P * IN_F), dtype=np.float32)
    sel8 = np.ascontiguousarray(
        np.tile(np.eye(P, dtype=np.float32), (8, 1)))            # [128, 16]
    w_mid = np.ascontiguousarray(
        np.concatenate([W[0:128, :], W[128:256, :]], axis=1), dtype=np.float32
    )
    wo = 0.25 * W_out
    w_out_t = np.ascontiguousarray(
        np.concatenate([wo[q * 128:(q + 1) * 128, :] for q in range(4)], axis=1),
        dtype=np.float32,
    )
    gb = np.ascontiguousarray(
        np.tile(np.concatenate([gamma, beta])[None, :], (BB, 1)), dtype=np.float32
    )
    ident8 = np.ascontiguousarray(np.eye(BB), dtype=np.float32)
    return w_pp, w_mid, w_out_t, gb, ident8, sel8


def kernel(**inputs):
    feature = np.asarray(inputs["feature"], dtype=np.float32)
    token = np.asarray(inputs["token"], dtype=np.float32)
    b_out = np.asarray(inputs["b_out"], dtype=np.float32)
    w_pp, w_mid, w_out_t, gb, ident8, sel8 = _prep_weights(
        np.asarray(inputs["W_patch"], dtype=np.float32),
        np.asarray(inputs["b_patch"], dtype=np.float32),
        np.asarray(inputs["W"], dtype=np.float32),
        np.asarray(inputs["W_out"], dtype=np.float32),
        b_out,
        np.asarray(inputs["gamma"], dtype=np.float32),
        np.asarray(inputs["beta"], dtype=np.float32),
    )
    W_full = np.asarray(inputs["W"], dtype=np.float32)
    W_out_full = np.asarray(inputs["W_out"], dtype=np.float32)
    b_patch_f = np.asarray(inputs["b_patch"], dtype=np.float32)
    bias_path = 0.25 * ((1024.0 * b_patch_f) @ W_full @ W_out_full)
    tok_adj = (0.75 * token + 0.25 * b_out[None, :] + bias_path[None, :]).astype(np.float32)
    nc = _get_program()
    in_maps = []
    for i in range(N_CORES):
        in_maps.append({
            "feature": np.ascontiguousarray(feature[i * BB:(i + 1) * BB]),
            "tok_adj": np.ascontiguousarray(tok_adj[i * BB:(i + 1) * BB]),
            "gb": gb,
            "sel8": sel8,
            "ident8": ident8,
            "w_pp": w_pp,
            "w_mid": w_mid,
            "w_out": w_out_t,
        })
    res = run_bass_kernel_spmd(nc, in_maps, list(range(N_CORES))).results
    return np.ascontiguousarray(
        np.concatenate([res[i]["out"] for i in range(N_CORES)], axis=0),
        dtype=np.float32,
    )


# revision 17
# speedup vs baseline: 1.3288x; 1.0600x over previous
"""Trainium2 Bass kernel for nn_AttentionBlock (gnn_message_passing).

Key simplification: the reference softmax is over a size-1 axis, so the
attention weights are exactly 1.0 and the patch einsum collapses to a sum
over all 1024 patches.  The whole module reduces to:

  S[b,c,p1,p2] = sum_{i,j} feature[b,c,16i+p1,16j+p2]        (201 MB read)
  u[b] = S[b] . W_patch (permuted) + 1024*b_patch            [256]
  v[b] = u[b] @ W                                            [512]
  y[b] = 0.25*(v[b] @ W_out + b_out) + 0.75*token[b]
  out[b] = layernorm(y[b]) * gamma + beta

Sharding: pure data parallel, batch 64 -> 8 cores x 8 batches.  Each core
streams its 25 MB feature shard (DMA-bound), reduces (k, j) on the vector
engine, collapses the remaining cross-partition i-dim with a 0/1 matmul,
and runs the small matmul tail in float32r (single-pass fp32, ~1e-4).
"""

import numpy as np
from contextlib import ExitStack

import concourse.bass as bass
import concourse.tile as tile
from concourse import bacc, mybir
from concourse.bass_utils import run_bass_kernel_spmd

F32 = mybir.dt.float32
F32R = mybir.dt.float32r

N_CORES = 8
B = 64
C = 3
H = 512
W_IMG = 512
P = 16                      # patch size
IN_F = 256
HD = 512
BB = B // N_CORES           # 8 batches per core
EPS = 1e-5


def _build_kernel_body(tc):
    nc = tc.nc
    feat = nc.dram_tensor("feature", [BB, C, H, W_IMG], F32R, kind="ExternalInput").ap()
    tok_adj = nc.dram_tensor("tok_adj", [BB, IN_F], F32, kind="ExternalInput").ap()
    gb = nc.dram_tensor("gb", [BB, 2 * IN_F], F32, kind="ExternalInput").ap()
    sel8 = nc.dram_tensor("sel8", [128, 16], F32R, kind="ExternalInput").ap()
    ident8 = nc.dram_tensor("ident8", [BB, BB], F32, kind="ExternalInput").ap()
    w_pp = nc.dram_tensor("w_pp", [16, C * 16 * IN_F], F32R, kind="ExternalInput").ap()
    w_mid = nc.dram_tensor("w_mid", [128, 2 * HD], F32R, kind="ExternalInput").ap()
    w_out = nc.dram_tensor("w_out", [128, 4 * IN_F], F32R, kind="ExternalInput").ap()
    out = nc.dram_tensor("out", [BB, IN_F], F32, kind="ExternalOutput").ap()

    with ExitStack() as ctx:
        mega = ctx.enter_context(tc.tile_pool(name="mega", bufs=12))
        small = ctx.enter_context(tc.tile_pool(name="small", bufs=1))
        work = ctx.enter_context(tc.tile_pool(name="work", bufs=2))
        psum = ctx.enter_context(tc.tile_pool(name="psum", bufs=4, space="PSUM"))
        psum_t = ctx.enter_context(tc.tile_pool(name="psum_t", bufs=1, space="PSUM"))
        psum_u = ctx.enter_context(tc.tile_pool(name="psum_u", bufs=1, space="PSUM"))
        psum_mm = ctx.enter_context(tc.tile_pool(name="psum_mm", bufs=1, space="PSUM"))

        # ---- constants / weights ----
        w_pp_t = small.tile([16, C * 16 * IN_F], F32R)
        nc.sync.dma_start(w_pp_t, w_pp)
        w_mid_t = small.tile([128, 2 * HD], F32R)
        nc.sync.dma_start(w_mid_t, w_mid)
        w_out_t = small.tile([128, 4 * IN_F], F32R)
        nc.sync.dma_start(w_out_t, w_out)
        gb_t = small.tile([BB, 2 * IN_F], F32)
        nc.sync.dma_start(gb_t, gb)
        tok_t = small.tile([BB, IN_F], F32)
        nc.sync.dma_start(tok_t, tok_adj)
        sel8_t = small.tile([128, 16], F32R)
        nc.sync.dma_start(sel8_t, sel8)
        ident8_t = small.tile([BB, BB], F32)
        nc.sync.dma_start(ident8_t, ident8)
        eps_t = small.tile([BB, 1], F32)
        nc.vector.memset(eps_t, EPS)

        # ---- stage 1 + 2 + 3, channel-major so the tail pipelines ----
        # per (b, c): one 3-D DMA, full 128 partitions, 1 MB (f32r):
        #   sbuf [r=128 rows (i8, p1), (k4, w512)];  row r_full = k*128 + r
        # PE collapses (i8, k) with four f32r sel8 matmuls accumulating in
        # PSUM [p1, (j, q)]; DVE then reduces j (strided) into st32_all.
        st32_all = small.tile([16, BB * C * 16], F32)
        u_ps = psum_u.tile([BB, IN_F], F32, tag="u")
        dma_engines = [nc.sync, nc.scalar]
        for c in range(C):
            for b in range(BB):
                idx = c * BB + b
                mt = mega.tile([128, 4 * W_IMG], F32R)
                src = feat[b, c].rearrange("(k r) w -> r k w", r=128)
                dma_engines[idx % 2].dma_start(
                    mt[:].rearrange("p (k w) -> p k w", w=W_IMG), src
                )
                # DVE folds k-pairs (f32r out), PE contracts i8 via sel8.
                # Alternate DVE-heavy and PE-heavy pairs so both engines stay
                # below the DMA delivery rate (no end-of-stream backlog).
                t01 = work.tile([128, W_IMG], F32R, tag="t01", bufs=6)
                nc.vector.tensor_add(t01, mt[:, 0:W_IMG], mt[:, W_IMG:2 * W_IMG])
                bc_ps = psum.tile([16, W_IMG], F32, tag="bc")
                if idx % 2 == 0:
                    t23 = work.tile([128, W_IMG], F32R, tag="t23", bufs=6)
                    nc.vector.tensor_add(t23, mt[:, 2 * W_IMG:3 * W_IMG],
                                         mt[:, 3 * W_IMG:4 * W_IMG])
                    nc.tensor.matmul(bc_ps, sel8_t, t01, start=True, stop=False)
                    nc.tensor.matmul(bc_ps, sel8_t, t23, start=False, stop=True)
                else:
                    nc.tensor.matmul(bc_ps, sel8_t, t01, start=True, stop=False)
                    nc.tensor.matmul(bc_ps, sel8_t,
                                     mt[:, 2 * W_IMG:3 * W_IMG],
                                     start=False, stop=False)
                    nc.tensor.matmul(bc_ps, sel8_t,
                                     mt[:, 3 * W_IMG:4 * W_IMG],
                                     start=False, stop=True)
                # bc_ps free index = j*16 + q ; reduce j, keep q
                mv = bc_ps[:].rearrange("p (j q) -> p q j", q=16)
                nc.vector.reduce_sum(
                    st32_all[:, (b * C + c) * 16:(b * C + c + 1) * 16],
                    mv,
                    axis=mybir.AxisListType.X,
                )

            # ---- per channel: cast the c-slice to f32r, u matmuls ----
            st_sb = small.tile([16, 128], F32R, tag=f"st{c}")
            stview = st32_all[:].rearrange("p (b c q) -> p b c q", c=C, q=16)
            nc.vector.tensor_copy(
                st_sb[:].rearrange("p (b q) -> p b q", q=16), stview[:, :, c, :]
            )

            stv = st_sb[:].rearrange("k (b q) -> k b q", q=16)
            for p2 in range(P):
                nc.tensor.matmul(
                    u_ps,
                    stv[:, :, p2],
                    w_pp_t[:, (c * P + p2) * IN_F:(c * P + p2 + 1) * IN_F],
                    start=(c == 0 and p2 == 0),
                    stop=(c == C - 1 and p2 == P - 1),
                )

        # ---- stage 4: transpose u to [256, 8] (bias folded into tok_adj) ----
        u_sb = work.tile([BB, IN_F], F32)
        nc.scalar.copy(u_sb, u_ps)

        uT_sb = work.tile([128, 2 * BB], F32R)
        for h in range(2):
            ut_ps = psum_t.tile([128, BB], F32, tag="tp2")
            nc.tensor.transpose(ut_ps, u_sb[:, h * 128:(h + 1) * 128], ident8_t)
            nc.vector.tensor_copy(uT_sb[:, h * BB:(h + 1) * BB], ut_ps)

        # ---- stage 5: v = u @ W ----
        v_ps = psum_mm.tile([BB, HD], F32, tag="v")
        for h in range(2):
            nc.tensor.matmul(
                v_ps,
                uT_sb[:, h * BB:(h + 1) * BB],
                w_mid_t[:, h * HD:(h + 1) * HD],
                start=(h == 0),
                stop=(h == 1),
            )
        v_sb = work.tile([BB, HD], F32)
        nc.scalar.copy(v_sb, v_ps)

        vT_sb = work.tile([128, 4 * BB], F32R)
        for q in range(4):
            vt_ps = psum_t.tile([128, BB], F32, tag="tp2")
            nc.tensor.transpose(vt_ps, v_sb[:, q * 128:(q + 1) * 128], ident8_t)
            nc.vector.tensor_copy(vT_sb[:, q * BB:(q + 1) * BB], vt_ps)

        # ---- stage 6: y = 0.25*v@W_out + (0.25*b_out + 0.75*token) ----
        y_ps = psum_mm.tile([BB, IN_F], F32, tag="y")
        for q in range(4):
            nc.tensor.matmul(
                y_ps,
                vT_sb[:, q * BB:(q + 1) * BB],
                w_out_t[:, q * IN_F:(q + 1) * IN_F],
                start=(q == 0),
                stop=(q == 3),
            )
        y_sb = work.tile([BB, IN_F], F32)
        nc.vector.tensor_add(y_sb, y_ps, tok_t)

        # ---- stage 7: layernorm ----
        stats = work.tile([BB, 6], F32)
        nc.vector.bn_stats(stats, y_sb)
        mv2 = work.tile([BB, 2], F32)
        nc.vector.bn_aggr(mv2, stats)
        std = work.tile([BB, 1], F32)
        nc.scalar.activation(std, mv2[:, 1:2], mybir.ActivationFunctionType.Sqrt,
                             bias=eps_t, scale=1.0)
        rstd = work.tile([BB, 1], F32)
        nc.vector.reciprocal(rstd, std)
        xm = work.tile([BB, IN_F], F32)
        nc.vector.tensor_scalar(xm, y_sb, mv2[:, 0:1], rstd,
                                op0=mybir.AluOpType.subtract,
                                op1=mybir.AluOpType.mult)
        out_sb = work.tile([BB, IN_F], F32)
        nc.vector.tensor_mul(out_sb, xm, gb_t[:, 0:IN_F])
        nc.vector.tensor_add(out_sb, out_sb, gb_t[:, IN_F:2 * IN_F])
        nc.sync.dma_start(out, out_sb)


_CACHE = {}


def _get_program():
    if "nc" not in _CACHE:
        nc = bacc.Bacc("TRN2", target_bir_lowering=False, debug=False,
                       num_devices=N_CORES)
        with tile.TileContext(nc) as tc:
            _build_kernel_body(tc)
        nc.compile()
        _CACHE["nc"] = nc
    return _CACHE["nc"]


def _prep_weights(W_patch, b_patch, W, W_out, b_out, gamma, beta):
    # w_pp[p1, (c, p2, f)] = W_patch[(p1*16+p2)*3 + c, f]
    wp4 = W_patch.reshape(P, P, C, IN_F).transpose(0, 2, 1, 3)   # [p1, c, p2, f]
    w_pp = np.ascontiguousarray(wp4.reshape(P, C * P * IN_F), dtype=np.float32)
    sel8 = np.ascontiguousarray(
        np.tile(np.eye(P, dtype=np.float32), (8, 1)))            # [128, 16]
    w_mid = np.ascontiguousarray(
        np.concatenate([W[0:128, :], W[128:256, :]], axis=1), dtype=np.float32
    )
    wo = 0.25 * W_out
    w_out_t = np.ascontiguousarray(
        np.concatenate([wo[q * 128:(q + 1) * 128, :] for q in range(4)], axis=1),
        dtype=np.float32,
    )
    gb = np.ascontiguousarray(
        np.tile(np.concatenate([gamma, beta])[None, :], (BB, 1)), dtype=np.float32
    )
    ident8 = np.ascontiguousarray(np.eye(BB), dtype=np.float32)
    return w_pp, w_mid, w_out_t, gb, ident8, sel8


def kernel(**inputs):
    feature = np.asarray(inputs["feature"], dtype=np.float32)
    token = np.asarray(inputs["token"], dtype=np.float32)
    b_out = np.asarray(inputs["b_out"], dtype=np.float32)
    w_pp, w_mid, w_out_t, gb, ident8, sel8 = _prep_weights(
        np.asarray(inputs["W_patch"], dtype=np.float32),
        np.asarray(inputs["b_patch"], dtype=np.float32),
        np.asarray(inputs["W"], dtype=np.float32),
        np.asarray(inputs["W_out"], dtype=np.float32),
        b_out,
        np.asarray(inputs["gamma"], dtype=np.float32),
        np.asarray(inputs["beta"], dtype=np.float32),
    )
    W_full = np.asarray(inputs["W"], dtype=np.float32)
    W_out_full = np.asarray(inputs["W_out"], dtype=np.float32)
    b_patch_f = np.asarray(inputs["b_patch"], dtype=np.float32)
    bias_path = 0.25 * ((1024.0 * b_patch_f) @ W_full @ W_out_full)
    tok_adj = (0.75 * token + 0.25 * b_out[None, :] + bias_path[None, :]).astype(np.float32)
    nc = _get_program()
    in_maps = []
    for i in range(N_CORES):
        in_maps.append({
            "feature": np.ascontiguousarray(feature[i * BB:(i + 1) * BB]),
            "tok_adj": np.ascontiguousarray(tok_adj[i * BB:(i + 1) * BB]),
            "gb": gb,
            "sel8": sel8,
            "ident8": ident8,
            "w_pp": w_pp,
            "w_mid": w_mid,
            "w_out": w_out_t,
        })
    res = run_bass_kernel_spmd(nc, in_maps, list(range(N_CORES))).results
    return np.ascontiguousarray(
        np.concatenate([res[i]["out"] for i in range(N_CORES)], axis=0),
        dtype=np.float32,
    )
